# revision 14
# baseline (speedup 1.0000x reference)
"""Trainium2 Bass kernel for AdvancedKANLayer.

Math (per reference):
  xn    = LayerNorm(x) * ln_w + ln_b           (eps=1e-5)
  base  = silu(xn) @ base_weight.T             [B,S,O]
  t     = tanh(xn)
  basis = cos(pi*k*t), k=1..8
  spl   = einsum('bsig,oig->bso', basis, spline_weight)
  out   = base + spl

Strategy: data-parallel over batch (8 cores, one batch entry each, no
collectives).  Per core the whole thing is one K=18432 GEMM:
  out[o, t] = sum_k W_all[k, o] * panel[k, t]
where panel rows are [silu(xn); cos(1*pi*t); ...; cos(8*pi*t)] per
I-chunk, generated on-chip.  cos(k*pi*t) is built from
c1 = cos(pi*t) = 1 - 2*sin(pi*t/2)^2 via Chebyshev product
identities on the VectorEngine (ScalarE Sin is only valid on [-pi,pi]).
Weights are pre-transposed/pre-tiled on the host, cast to bf16; matmul
runs bf16 with f32 PSUM accumulation.

K-step order is ic-major: step s = ic*9 + m (m=0 silu, m=1..8 cos_m).
Each chunk's GEMM runs as NPASS O-passes (OTP o-tiles each, one PSUM
bank per o-tile, o-tile innermost) so panel tiles are released
progressively during the LAST pass; generation of the next chunk's
panel is interleaved into that pass, keeping TensorE busy (and HAM
warm) across chunk boundaries.
"""

import math

import numpy as np
import ml_dtypes

import concourse.bass as bass
import concourse.mybir as mybir
import concourse.tile as tile
from concourse import bacc
from concourse import masks
from concourse.bass import ds, ts
from concourse.bass_utils import run_bass_kernel_spmd

F32 = mybir.dt.float32
BF16 = mybir.dt.bfloat16
AF = mybir.ActivationFunctionType
ALU = mybir.AluOpType

EPS = 1e-5

# geometry (full problem, per core)
B = 8
T = 2048          # tokens per core (= S, one batch entry per core)
I = 2048          # input dim
O = 2048          # output dim
G = 8             # cos harmonics
TCH = 512         # token chunk (matmul N)
NM = G + 1        # 9 panel row-groups per ic (silu + 8 cos)
OTP = 4           # o-tiles per O-pass (one PSUM bank each)


def build_nc(nT=T, nI=I, nO=O, tch=TCH):
    nch = nT // tch
    nic = nI // 128
    nk = nic * NM
    n_ot = nO // 128
    ntt = tch // 128          # token-tiles per chunk
    otp = min(OTP, n_ot)
    npass = n_ot // otp

    nc = bacc.Bacc("TRN2", target_bir_lowering=False, debug=False)
    x_ext = nc.declare_dram_parameter("x", [nT, nI], F32, isOutput=False)
    lnw_ext = nc.declare_dram_parameter("lnw", [nI], F32, isOutput=False)
    lnb_ext = nc.declare_dram_parameter("lnb", [nI], F32, isOutput=False)
    wt_ext = nc.declare_dram_parameter(
        "wt", [npass, nk, 128, otp, 128], BF16, isOutput=False
    )
    out_ext = nc.declare_dram_parameter("out", [nO, nT], F32, isOutput=True)

    with tile.TileContext(nc) as tc:
        with (
            tc.tile_pool(name="consts", bufs=1) as consts,
            tc.tile_pool(name="xp", bufs=1) as xpool,
            tc.tile_pool(name="statp", bufs=2) as statp,
            tc.tile_pool(name="genp", bufs=2) as genp,
            tc.tile_pool(name="ladp", bufs=1) as ladp,
            tc.tile_pool(name="panelp", bufs=1) as panelp,
            tc.tile_pool(name="wp", bufs=6) as wp,
            tc.tile_pool(name="stgp", bufs=2) as stgp,
            tc.tile_pool(name="tpps", bufs=2, space="PSUM") as tpps,
            tc.tile_pool(name="mmps", bufs=1, space="PSUM") as mmps,
        ):
            identity = consts.tile([128, 128], F32)
            masks.make_identity(nc, identity[:])
            lnw_sb = consts.tile([128, nic], F32)
            nc.sync.dma_start(lnw_sb[:], lnw_ext.rearrange("(f p) -> p f", p=128))
            lnb_sb = consts.tile([128, nic], F32)
            nc.sync.dma_start(lnb_sb[:], lnb_ext.rearrange("(f p) -> p f", p=128))
            eps_sb = consts.tile([128, 1], F32)
            nc.gpsimd.memset(eps_sb[:], EPS)

            # per-chunk normalized-x tiles from preamble
            state = {}

            def preamble(c):
                """x DMA + LN stats + normalize (tokens on partitions)."""
                stats = statp.tile([128, ntt, 2], F32, tag="stats")
                xts = []
                for j in range(ntt):
                    xt = xpool.tile([128, nI], F32, tag=f"xt{j}",
                                    name=f"xt_{c}_{j}")
                    nc.sync.dma_start(xt[:], x_ext[ds((c * ntt + j) * 128, 128), :])
                    bn6 = statp.tile([128, 4, 6], F32, tag="bn6")
                    for q in range(4):
                        nc.vector.bn_stats(
                            bn6[:, q, :], xt[:, ds(q * (nI // 4), nI // 4)]
                        )
                    nc.vector.bn_aggr(stats[:, j, :], bn6[:])
                    xts.append(xt)
                std = statp.tile([128, ntt], F32, tag="std")
                nc.scalar.activation(std[:], stats[:, :, 1], AF.Sqrt, bias=eps_sb[:])
                istd = statp.tile([128, ntt], F32, tag="istd")
                nc.vector.reciprocal(istd[:], std[:])
                nmi = statp.tile([128, ntt], F32, tag="nmi")
                nc.vector.scalar_tensor_tensor(
                    nmi[:], stats[:, :, 0], -1.0, istd[:], ALU.mult, ALU.mult
                )
                for j in range(ntt):
                    # normalize in place: xn = (x - mu) * istd
                    nc.scalar.activation(
                        xts[j][:], xts[j][:], AF.Identity,
                        bias=nmi[:, j : j + 1], scale=istd[:, j : j + 1],
                    )
                state[c] = xts

            def gen_ic(c, ic, ptiles):
                """transpose + tanh/silu + cheb ladder for I-chunk ic."""
                xnts = state[c]
                tp = tpps.tile([128, tch], F32, tag="tp", name=f"tp_{c}_{ic}")
                for j in range(ntt):
                    nc.tensor.transpose(
                        tp[:, ts(j, 128)], xnts[j][:, ts(ic, 128)], identity[:]
                    )
                lw = lnw_sb[:, ic : ic + 1]
                lb = lnb_sb[:, ic : ic + 1]

                def pt(m):
                    s = ic * NM + m
                    t_ = panelp.tile(
                        [128, tch], BF16, tag=f"p{s:03d}", name=f"panel_{c}_{s:03d}"
                    )
                    ptiles[s] = t_
                    return t_

                def lad(tag):
                    return ladp.tile(
                        [128, tch], F32, tag=tag, name=f"lad_{tag}_{c}_{ic}"
                    )

                def stt(out, a, s, b):
                    nc.vector.scalar_tensor_tensor(
                        out[:], a[:], s, b[:], ALU.mult, ALU.mult
                    )

                th = genp.tile([128, tch], F32, tag="th", name=f"th_{c}_{ic}")
                nc.scalar.activation(th[:], tp[:], AF.Tanh, bias=lb, scale=lw)
                nc.scalar.activation(pt(0)[:], tp[:], AF.Silu, bias=lb, scale=lw)
                sh = genp.tile([128, tch], F32, tag="sh", name=f"sh_{c}_{ic}")
                nc.scalar.activation(sh[:], th[:], AF.Sin, scale=math.pi / 2)

                # c1 = 1 - 2*sh^2
                u = lad("u")
                stt(u, sh, -2.0, sh)
                c1 = lad("c1")
                nc.vector.tensor_scalar_add(c1[:], u[:], 1.0)
                sq1 = lad("sq")
                nc.scalar.square(sq1[:], c1[:])
                c2 = lad("c2")
                nc.vector.tensor_scalar(c2[:], sq1[:], 2.0, -1.0, ALU.mult, ALU.add)
                u3 = lad("u")
                stt(u3, c2, 2.0, c1)
                c3 = lad("c3")
                nc.vector.tensor_sub(c3[:], u3[:], c1[:])
                sq2 = lad("sq")
                nc.scalar.square(sq2[:], c2[:])
                c4 = lad("c4")
                nc.vector.tensor_scalar(c4[:], sq2[:], 2.0, -1.0, ALU.mult, ALU.add)
                nc.scalar.copy(pt(1)[:], c1[:])
                nc.scalar.copy(pt(2)[:], c2[:])
                nc.vector.tensor_copy(pt(3)[:], c3[:])
                nc.vector.tensor_copy(pt(4)[:], c4[:])
                u5 = lad("u")
                stt(u5, c3, 2.0, c2)
                nc.vector.tensor_sub(pt(5)[:], u5[:], c1[:])
                sq3 = lad("sq")
                nc.scalar.square(sq3[:], c3[:])
                nc.vector.tensor_scalar(pt(6)[:], sq3[:], 2.0, -1.0, ALU.mult, ALU.add)
                u7 = lad("u")
                stt(u7, c4, 2.0, c3)
                nc.vector.tensor_sub(pt(7)[:], u7[:], c1[:])
                sq4 = lad("sq")
                nc.scalar.square(sq4[:], c4[:])
                nc.vector.tensor_scalar(pt(8)[:], sq4[:], 2.0, -1.0, ALU.mult, ALU.add)

            def mm_pass(c, p, ptiles, gen_next=None, next_ptiles=None):
                """One O-pass: o-tiles p*otp..p*otp+otp-1, o-tile innermost.
                On the last pass, interleave generation of the next chunk's
                panel as its tiles are released."""
                pss = [
                    mmps.tile([128, tch], F32, tag=f"ps{j}", name=f"ps_{c}_{p}_{j}")
                    for j in range(otp)
                ]
                for s in range(nk):
                    wg = wp.tile([128, otp, 128], BF16, tag="wg",
                                 name=f"wg_{c}_{p}_{s}")
                    nc.sync.dma_start(wg[:], wt_ext[p, s])
                    for j in range(otp):
                        nc.tensor.matmul(
                            pss[j][:], wg[:, j, :], ptiles[s][:],
                            start=(s == 0), stop=(s == nk - 1),
                        )
                    if gen_next is not None and s % NM == NM - 1:
                        gen_ic(gen_next, s // NM, next_ptiles)
                for j in range(otp):
                    ot = p * otp + j
                    stg = stgp.tile([128, tch], F32, tag="stg",
                                    name=f"stg_{c}_{p}_{j}")
                    nc.vector.tensor_copy(stg[:], pss[j][:])
                    nc.sync.dma_start(
                        out_ext[ds(ot * 128, 128), ds(c * tch, tch)], stg[:]
                    )

            # ---- schedule ----
            preamble(0)
            ptiles = [None] * nk
            for ic in range(nic):
                gen_ic(0, ic, ptiles)
            for c in range(nch):
                if c + 1 < nch:
                    preamble(c + 1)
                next_ptiles = [None] * nk
                for p in range(npass):
                    last = p == npass - 1 and c + 1 < nch
                    mm_pass(
                        c, p, ptiles,
                        gen_next=(c + 1) if last else None,
                        next_ptiles=next_ptiles if last else None,
                    )
                ptiles = next_ptiles

    nc.compile()
    return nc


def prep_weights(base_weight, spline_weight, nO=O, nI=I):
    """Host-side: bf16 W_all in ic-major k-step order, tiled per O-pass:
    wt[pass, s, k_in, j, o_in] with o-tile = pass*otp + j."""
    nic = nI // 128
    nk = nic * NM
    n_ot = nO // 128
    otp = min(OTP, n_ot)
    npass = n_ot // otp
    w = np.empty((NM, nI, nO), np.float32)
    w[0] = base_weight.T                      # [i, o]
    for g in range(G):
        w[1 + g] = spline_weight[:, :, g].T   # [i, o]
    # m-major [9, nic, 128, nO] -> ic-major [nic, 9, 128, nO] -> [nk, 128, nO]
    w = w.reshape(NM, nic, 128, nO).transpose(1, 0, 2, 3).reshape(nk, 128, nO)
    # [s, k_in, npass, otp, o_in] -> [npass, s, k_in, otp, o_in]
    w = w.reshape(nk, 128, npass, otp, 128).transpose(2, 0, 1, 3, 4)
    return np.ascontiguousarray(w.astype(ml_dtypes.bfloat16))


_NC_CACHE = {}


def _get_nc():
    if "nc" not in _NC_CACHE:
        _NC_CACHE["nc"] = build_nc()
    return _NC_CACHE["nc"]


def kernel(x, ln_weight, ln_bias, base_weight, spline_weight):
    x = np.asarray(x, np.float32)
    ln_weight = np.asarray(ln_weight, np.float32)
    ln_bias = np.asarray(ln_bias, np.float32)
    wt = prep_weights(np.asarray(base_weight, np.float32),
                      np.asarray(spline_weight, np.float32))
    nc = _get_nc()
    in_maps = [
        {
            "x": np.ascontiguousarray(x[b]),
            "lnw": ln_weight,
            "lnb": ln_bias,
            "wt": wt,
        }
        for b in range(B)
    ]
    res = run_bass_kernel_spmd(nc, in_maps, core_ids=list(range(B)))
    out = np.stack([res.results[b]["out"].T for b in range(B)])
    return np.ascontiguousarray(out.astype(np.float32))
